# revision 42
# baseline (speedup 1.0000x reference)
"""Bass/Tile TRN2 kernel for nn_BayesHead (projected single-head attention,
near-causal mask tril(diag=1), double 1/sqrt(64) scaling).

Strategy (8 NeuronCores, pure data-parallel SPMD — no collectives):
  - core j handles batch b = j//2 with key-parity p = j%2.
  - Each core projects ALL 4096 queries of its batch, and its HALF of the
    keys/values (interleaved 128-row blocks: global block g = 2*sigma + p).
  - Flash-style partial softmax without max-subtraction (scores are in
    [-1,1] after the 1/64 scaling, so exp is safe): each core produces
    O_p[h, t] = sum_{s in its keys, s <= t+1} exp(S) * V[s, h] plus a
    denominator row (ones-column trick).  The host sums the two partials
    per batch and normalizes.

v2 scheduling (vs the 109us baseline):
  - DRAM inputs pre-laid-out as [128, ct, t] so one dma_start per 512-col
    chunk lands contiguously; chunks issued in exact consumption order so
    all 16 DMA engines run from t~0 and compute starts at ~4us.
  - Mask tensors built on the (otherwise idle) GPSIMD engine so the DVE
    stream never blocks early PSUM-evacuation copies.
  - Projections interleaved with attention tiles so the PE stays
    continuously busy (HAM ramps to 2.4 GHz) while the ACT engine chews
    the exp stream in parallel.
  - The last s-tile of each query tile is >99% masked (only its first key
    is visible, to the last query): scores/exp/mask/PV are trimmed to the
    final 128 columns there.
"""

import numpy as np
from contextlib import ExitStack

import concourse.bass as bass
import concourse.mybir as mybir
import concourse.tile as tile
from concourse import bacc
from concourse.bass import ts
from concourse.bass_utils import run_bass_kernel_spmd

B, T, C, H = 4, 4096, 1024, 64
NCORES = 8
TQ = 512                       # query-tile width
NQT = T // TQ                  # 8 query tiles
NSB = (T // 2) // 128          # 16 local key tiles (128 rows each)
NCT = C // 128                 # 8 contraction tiles
TH = T // 2
# s-tile capacity per query tile (identical for both parities; covers causal
# reach ceil((4i+5)/2), capped at the 16 local tiles)
CAPS = [min(NSB, 2 * i + 3) for i in range(NQT)]
MASK_FROM = [2 * i for i in range(NQT)]  # sigma >= 2i may cross the diagonal
# The mask for tile (i, s) depends only on e = 2s - 4i in {0, 2, 4}:
# thr = 128*(2s+p) + r - 512i - 1 = 128*e + 128*p + r - 1.  Three masks total.
M_IDX = {(i, s): (2 * s - 4 * i) // 2
         for i in range(NQT) for s in range(MASK_FROM[i], CAPS[i])}
N_MASKED = 3
W0 = 480                       # live-column window start for singleton s-tiles
# (p0 singleton has exactly 1 live col (511); p1 singleton is fully dead)
FP = mybir.dt.float16
F32 = mybir.dt.float32
SCALE = 1.0 / H                # (H**-0.5) applied twice


def build_bass():
    nc = bacc.Bacc("TRN2", target_bir_lowering=False, num_devices=NCORES)
    # DRAM layouts are pre-transposed on host and chunk-major:
    # x[p, chunk, ct, col] = x.T[128*ct+p, 512*chunk+col], so each 512-col
    # chunk DMA moves 8KB contiguous per partition (128 fat descriptors)
    qT = nc.declare_dram_parameter("qT", [128, NQT, NCT, 512], FP, isOutput=False)
    kT = nc.declare_dram_parameter("kT", [128, NQT // 2, NCT, 512], FP, isOutput=False)
    vT = nc.declare_dram_parameter("vT", [128, NQT // 2, NCT, 512], FP, isOutput=False)
    # wq|wk|wv merged into one DMA; iota|ident|thr merged into one (all fp16)
    w3 = nc.declare_dram_parameter("w3", [128, 3, NCT, H], FP, isOutput=False)
    misc = nc.declare_dram_parameter("misc", [128, TQ + 64 + N_MASKED], FP,
                                     isOutput=False)
    out = nc.declare_dram_parameter("out", [H + 1, T], F32, isOutput=True)

    with ExitStack() as ctx:
        tc = ctx.enter_context(tile.TileContext(nc))
        singles = ctx.enter_context(tc.tile_pool(name="singles", bufs=1))
        pt_pool = ctx.enter_context(tc.tile_pool(name="pt", bufs=6))
        outsb_pool = ctx.enter_context(tc.tile_pool(name="outsb", bufs=6))
        stage_pool = ctx.enter_context(tc.tile_pool(name="stage", bufs=2))
        psum_s = ctx.enter_context(tc.tile_pool(name="psum_s", bufs=3, space="PSUM"))
        psum_o = ctx.enter_context(tc.tile_pool(name="psum_o", bufs=2, space="PSUM"))

        # SBUF-resident tiles
        misc_sb = singles.tile([128, TQ + 64 + N_MASKED], FP)
        iota_sb = misc_sb[:, 0:TQ]
        id_sb = misc_sb[0:64, TQ:TQ + 64]
        thr_sb = singles.tile([128, N_MASKED], F32)
        w3_sb = singles.tile([128, 3, NCT, H], FP)
        wq_sb = w3_sb[:, 0]
        wk_sb = w3_sb[:, 1]
        wv_sb = w3_sb[:, 2]
        q_sb = singles.tile([128, NQT, NCT, 512], FP)
        k_sb = singles.tile([128, NQT // 2, NCT, 512], FP)
        v_sb = singles.tile([128, NQT // 2, NCT, 512], FP)

        qp_sb = singles.tile([128, T], FP)        # Q^T [h, t], dup on parts 64-127
        kp_sb = singles.tile([128, TH], FP)       # K^T [h, s], dup on parts 64-127
        va_sb = singles.tile([128, NSB, H + 1], FP)  # V rows [s, h] + ones col
        masks_sb = singles.tile([128, N_MASKED, TQ], FP)

        # ---- DMA issue stream (sync engine), deadline order ----
        # Arrival pacing is ~2.85us/MB; the exp (ACT) stream is paced by q_i
        # arrivals early on, so q chunks go as early as k/v deadlines allow.
        def dq(c):
            nc.sync.dma_start(out=q_sb[:, c, :, :], in_=qT[:, c, :, :])

        def dk(c, c0=0, c1=512):
            nc.sync.dma_start(out=k_sb[:, c, :, c0:c1], in_=kT[:, c, :, c0:c1])

        def dv(c, c0=0, c1=512):
            nc.sync.dma_start(out=v_sb[:, c, :, c0:c1], in_=vT[:, c, :, c0:c1])

        nc.sync.dma_start(out=w3_sb, in_=w3[:, :, :, :])
        dk(0)
        dq(0)
        dq(1)
        nc.sync.dma_start(out=misc_sb, in_=misc[:, :])
        dv(0)
        dk(1)
        dq(2)
        dv(1)
        dq(3)
        dk(2)
        dq(4)
        dv(2)
        dq(5)
        dk(3)
        dq(6)
        dq(7)
        dv(3)

        # ones column for the softmax denominator
        nc.vector.memset(va_sb[:, :, H:H + 1], 1.0)

        # PE warm-up: the first ~10us are DMA-dead (preamble + q0/k0 landing);
        # keep the PE busy on junk matmuls so HAM reaches K=8/8 (2.4 GHz)
        # before the first projection instead of ramping mid-projection.
        warm_sb = singles.tile([128, 576], FP)
        nc.vector.memset(warm_sb, 0.25)
        warm_ps = psum_s.tile([128, 512], F32, tag="ps")
        for _ in range(14):
            nc.tensor.matmul(warm_ps, warm_sb[:, 0:128], warm_sb[:, 64:576],
                             start=True, stop=True, skip_group_check=True)



        def build_masks():
            # Only 3 distinct masks exist (e = 2s-4i in {0,2,4}); build once.
            nc.vector.tensor_copy(            # fp16 -> f32 (is_ge wants f32)
                thr_sb, misc_sb[:, TQ + 64:TQ + 64 + N_MASKED])
            for m in range(N_MASKED):
                nc.vector.tensor_scalar(
                    masks_sb[:, m, :], iota_sb[:, :], thr_sb[:, m:m + 1],
                    None, mybir.AluOpType.is_ge)

        def q_proj(tq):
            pq = psum_s.tile([128, 512], F32, tag="ps")
            for ct in range(NCT):
                nc.tensor.matmul(pq[0:64, :], wq_sb[:, ct, :],
                                 q_sb[:, tq, ct, :], tile_position=(0, 0),
                                 start=(ct == 0), stop=(ct == NCT - 1))
                nc.tensor.matmul(pq[64:128, :], wq_sb[:, ct, :],
                                 q_sb[:, tq, ct, :], tile_position=(0, 64),
                                 start=(ct == 0), stop=(ct == NCT - 1),
                                 skip_group_check=True)
            nc.vector.tensor_copy(qp_sb[:, ts(tq, 512)], pq)

        def q_proj_pair(ta, tb):
            # Two q-tiles share one slot as independent col tiles; the
            # partition-dup (needed by the row-tiled scores) comes from DVE
            # copies instead of a redundant second matmul.
            pq = psum_s.tile([128, 512], F32, tag="ps")
            for ct in range(NCT):
                nc.tensor.matmul(pq[0:64, :], wq_sb[:, ct, :],
                                 q_sb[:, ta, ct, :], tile_position=(0, 0),
                                 start=(ct == 0), stop=(ct == NCT - 1))
                nc.tensor.matmul(pq[64:128, :], wq_sb[:, ct, :],
                                 q_sb[:, tb, ct, :], tile_position=(0, 64),
                                 start=(ct == 0), stop=(ct == NCT - 1),
                                 skip_group_check=True)
            nc.vector.tensor_copy(qp_sb[0:64, ts(ta, 512)], pq[0:64, :])
            nc.vector.tensor_copy(qp_sb[64:128, ts(ta, 512)], pq[0:64, :])
            nc.vector.tensor_copy(qp_sb[0:64, ts(tb, 512)], pq[64:128, :])
            nc.vector.tensor_copy(qp_sb[64:128, ts(tb, 512)], pq[64:128, :])

        def k_proj(c4, c0=0, c1=512):
            pk = psum_s.tile([128, 512], F32, tag="ps")
            for ct in range(NCT):
                nc.tensor.matmul(pk[0:64, c0:c1], wk_sb[:, ct, :],
                                 k_sb[:, c4, ct, c0:c1], tile_position=(0, 0),
                                 start=(ct == 0), stop=(ct == NCT - 1))
                nc.tensor.matmul(pk[64:128, c0:c1], wk_sb[:, ct, :],
                                 k_sb[:, c4, ct, c0:c1], tile_position=(0, 64),
                                 start=(ct == 0), stop=(ct == NCT - 1),
                                 skip_group_check=True)
            nc.vector.tensor_copy(kp_sb[:, 512 * c4 + c0:512 * c4 + c1],
                                  pk[:, c0:c1])

        def v_proj(c4, j0=0, j1=4):
            cols = slice(128 * j0, 128 * j1)
            pv = psum_s.tile([64, 512], F32, tag="ps")
            for ct in range(NCT):
                nc.tensor.matmul(pv[:, cols], wv_sb[:, ct, :],
                                 v_sb[:, c4, ct, cols],
                                 start=(ct == 0), stop=(ct == NCT - 1))
            vt_stage = stage_pool.tile([64, 512], FP)
            nc.vector.tensor_copy(vt_stage[:, cols], pv[:, cols])
            for j in range(j0, j1):
                sig = c4 * 4 + j
                ptr = psum_o.tile([128, H], FP, tag="oacc")
                nc.tensor.transpose(ptr, vt_stage[:, ts(j, 128)], id_sb)
                nc.vector.tensor_copy(va_sb[:, sig, 0:H], ptr)

        # Global PV pipeline: entries are (emit_fn, pre_fn) closures; the PV
        # stream lags the global scores stream by LAG groups, crossing
        # attention boundaries so the PE never idles on an exp boundary.
        LAG = 4
        PEND = []

        def pump():
            emit_fn, pre_fn = PEND.pop(0)
            if pre_fn is not None:
                pre_fn()           # v-projection feeding this PV
            emit_fn()

        def attention(i, mid=None, mid_leads=None, pre_flush=None):
            cap = CAPS[i]
            po = psum_o.tile([H + 1, 512], F32, tag="oacc")
            # group list: leading full pairs, then the diagonal pair (needs
            # no new k-projection), then the nearly-dead singleton (trimmed
            # to cols [W0,512)) last — its k-projection (mid_leads) overlaps
            # the diagonal pair's exp.  The first PV writing po is a
            # full-width pair (PSUM zero-region start semantics).
            groups = []
            lead = cap if i == 7 else 2 * i + 2
            for g0 in range(0, lead, 2):
                groups.append(("pair", g0))
            if i < 7:
                groups.append(("single", cap - 1))
            def emit_s(kind, g0):
                # scores matmuls + exp + mask for one group; returns pt handle
                if kind == "pair":
                    ps = psum_s.tile([128, 1024], F32, tag="ps")
                    for g in (0, 1):
                        sig = g0 + g
                        nc.tensor.matmul(ps[:, ts(g, 512)],
                                         kp_sb[ts(g, 64), ts(sig, 128)],
                                         qp_sb[ts(g, 64), ts(i, 512)],
                                         tile_position=(64 * g, 0),
                                         start=True, stop=True)
                    pt = pt_pool.tile([128, 1024], FP)
                    nc.scalar.activation(pt, ps,
                                         mybir.ActivationFunctionType.Exp,
                                         scale=SCALE)
                    for g in (0, 1):
                        sig = g0 + g
                        if sig >= MASK_FROM[i]:
                            m = M_IDX[(i, sig)]
                            nc.vector.tensor_mul(pt[:, ts(g, 512)],
                                                 pt[:, ts(g, 512)],
                                                 masks_sb[:, m, :])
                else:  # singleton: full-width scores (PSUM zero-region rule),
                    # but exp/mask/PV trimmed to the live cols [W0:512)
                    sig = g0
                    m = M_IDX[(i, sig)]
                    ps = psum_s.tile([128, 512], F32, tag="ps")
                    nc.tensor.matmul(ps,
                                     kp_sb[0:64, ts(sig, 128)],
                                     qp_sb[0:64, ts(i, 512)],
                                     tile_position=(0, 0), start=True, stop=True)
                    pt = pt_pool.tile([128, 512], FP)
                    nc.scalar.activation(pt[:, W0:512], ps[:, W0:512],
                                         mybir.ActivationFunctionType.Exp,
                                         scale=SCALE)
                    nc.vector.tensor_mul(pt[:, W0:512], pt[:, W0:512],
                                         masks_sb[:, m, W0:512])
                return pt

            state = {"first": True}

            def emit_pv(kind, g0, pt, last_grp):
                if kind == "pair":
                    for g in (0, 1):
                        sig = g0 + g
                        nc.tensor.matmul(po, va_sb[:, sig, :], pt[:, ts(g, 512)],
                                         start=state["first"],
                                         stop=(last_grp and g == 1))
                        state["first"] = False
                else:
                    nc.tensor.matmul(po[:, W0:512], va_sb[:, g0, :],
                                     pt[:, W0:512], start=state["first"],
                                     stop=last_grp)
                    state["first"] = False

            def finalize():
                osb = outsb_pool.tile([H + 1, 512], F32)
                nc.vector.tensor_copy(osb, po)
                # gpsimd SWDGE ring: keeps the sync HWDGE ring free for the
                # input stream (ring-full blocking serializes transfers)
                nc.gpsimd.dma_start(out=out[:, ts(i, 512)], in_=osb)

            n_groups = len(groups)
            for gi, (kind, g0) in enumerate(groups):
                if kind == "single" and mid_leads is not None:
                    # k-projection feeding this singleton (its DMA chunk
                    # arrives later than the lead pairs' data)
                    mid_leads()
                pt = emit_s(kind, g0)
                last = (gi == n_groups - 1)
                pre = pre_flush if gi == 0 else None

                def entry(kind=kind, g0=g0, pt=pt, last=last):
                    emit_pv(kind, g0, pt, last_grp=last)
                    if last:
                        finalize()
                PEND.append((entry, pre))
                while len(PEND) > LAG:
                    pump()
            if mid is not None:
                # next tile's projections go here so the PE chews them while
                # the ACT engine finishes this tile's trailing exps
                mid()

        # ---- compute schedule: deadline-aligned with the DMA stream ----
        # attention(i) consumes qp_i, kp s-tiles <= 2i+2, va s-tiles <= 2i+2.
        # Each attention's mid-hook carries upcoming projections so they
        # overlap that tile's trailing exp stream on the ACT engine.
        build_masks()
        k_proj(0)                  # s0-3
        q_proj(0)
        attention(0, mid=lambda: q_proj(1), pre_flush=lambda: v_proj(0))
        attention(1, mid_leads=lambda: k_proj(1), mid=lambda: q_proj(2),
                  pre_flush=lambda: v_proj(1, 0, 2))
        attention(2, mid=lambda: q_proj(3),
                  pre_flush=lambda: v_proj(1, 2, 4))
        attention(3, mid_leads=lambda: k_proj(2), mid=lambda: q_proj(4),
                  pre_flush=lambda: v_proj(2, 0, 2))
        attention(4, mid=lambda: q_proj(5),
                  pre_flush=lambda: v_proj(2, 2, 4))
        attention(5, mid_leads=lambda: k_proj(3), mid=lambda: q_proj(6),
                  pre_flush=lambda: v_proj(3, 0, 2))
        attention(6, mid=lambda: q_proj(7),
                  pre_flush=lambda: v_proj(3, 2, 4))
        attention(7)
        while PEND:                # drain the tail of the global PV pipeline
            pump()

    nc.compile()
    return nc


_NC = None


def _get_nc():
    global _NC
    if _NC is None:
        _NC = build_bass()
    return _NC


def _prep_core_inputs(q, k, v, Wq, Wk, Wv):
    f2 = np.float16

    def wprep(W):
        # SBUF layout [p, ct, h] = W.T[ct*128+p, h]
        return np.ascontiguousarray(W.T.reshape(NCT, 128, H).transpose(1, 0, 2)).astype(f2)

    def xprep(x):
        # [p, chunk, ct, col] = x.T[128*ct+p, 512*chunk+col]
        xt = x.T.astype(f2)                       # [C, T']
        nch = xt.shape[1] // 512
        return np.ascontiguousarray(
            xt.reshape(NCT, 128, nch, 512).transpose(1, 2, 0, 3))

    w3_h = np.ascontiguousarray(
        np.stack([wprep(Wq), wprep(Wk), wprep(Wv)], axis=1))

    r = np.arange(128)
    in_maps = []
    for j in range(NCORES):
        b, p = j // 2, j % 2
        rows = (np.arange(TH) // 128) * 256 + p * 128 + (np.arange(TH) % 128)
        qT_h = xprep(q[b])
        kT_h = xprep(k[b][rows])
        vT_h = xprep(v[b][rows])
        misc_h = np.zeros((128, TQ + 64 + N_MASKED), f2)
        misc_h[:, 0:TQ] = np.arange(TQ, dtype=np.float32)[None, :]
        misc_h[0:64, TQ:TQ + 64] = np.eye(64, dtype=f2)
        for m in range(N_MASKED):
            misc_h[:, TQ + 64 + m] = (256 * m + 128 * p + r - 1).astype(f2)
        in_maps.append({
            "qT": qT_h, "kT": kT_h, "vT": vT_h,
            "w3": w3_h, "misc": misc_h,
        })
    return in_maps


def _run(inputs, trace=False, trace_kwargs=None):
    nc = _get_nc()
    in_maps = _prep_core_inputs(
        inputs["q"], inputs["k"], inputs["v"],
        inputs["Wq"], inputs["Wk"], inputs["Wv"])
    res = run_bass_kernel_spmd(nc, in_maps, list(range(NCORES)), trace=trace,
                               **(trace_kwargs or {}))
    outs = [res.results[j]["out"] for j in range(NCORES)]
    y = np.empty((B, T, H), np.float32)
    for b in range(B):
        s = outs[2 * b] + outs[2 * b + 1]      # [H+1, T]
        y[b] = (s[:H] / s[H:H + 1]).T
    return y, res


def kernel(q, k, v, Wq, Wk, Wv):
    y, _ = _run({"q": np.asarray(q), "k": np.asarray(k), "v": np.asarray(v),
                 "Wq": np.asarray(Wq), "Wk": np.asarray(Wk), "Wv": np.asarray(Wv)})
    return y



# revision 45
# speedup vs baseline: 1.0062x; 1.0062x over previous
"""Bass/Tile TRN2 kernel for nn_BayesHead (projected single-head attention,
near-causal mask tril(diag=1), double 1/sqrt(64) scaling).

Strategy (8 NeuronCores, pure data-parallel SPMD — no collectives):
  - core j handles batch b = j//2 with key-parity p = j%2.
  - Each core projects ALL 4096 queries of its batch, and its HALF of the
    keys/values (interleaved 128-row blocks: global block g = 2*sigma + p).
  - Flash-style partial softmax without max-subtraction (scores are in
    [-1,1] after the 1/64 scaling, so exp is safe): each core produces
    O_p[h, t] = sum_{s in its keys, s <= t+1} exp(S) * V[s, h] plus a
    denominator row (ones-column trick).  The host sums the two partials
    per batch and normalizes.

v2 scheduling (vs the 109us baseline):
  - DRAM inputs pre-laid-out as [128, ct, t] so one dma_start per 512-col
    chunk lands contiguously; chunks issued in exact consumption order so
    all 16 DMA engines run from t~0 and compute starts at ~4us.
  - Mask tensors built on the (otherwise idle) GPSIMD engine so the DVE
    stream never blocks early PSUM-evacuation copies.
  - Projections interleaved with attention tiles so the PE stays
    continuously busy (HAM ramps to 2.4 GHz) while the ACT engine chews
    the exp stream in parallel.
  - The last s-tile of each query tile is >99% masked (only its first key
    is visible, to the last query): scores/exp/mask/PV are trimmed to the
    final 128 columns there.
"""

import numpy as np
from contextlib import ExitStack

import concourse.bass as bass
import concourse.mybir as mybir
import concourse.tile as tile
from concourse import bacc
from concourse.bass import ts
from concourse.bass_utils import run_bass_kernel_spmd

B, T, C, H = 4, 4096, 1024, 64
NCORES = 8
TQ = 512                       # query-tile width
NQT = T // TQ                  # 8 query tiles
NSB = (T // 2) // 128          # 16 local key tiles (128 rows each)
NCT = C // 128                 # 8 contraction tiles
TH = T // 2
# s-tile capacity per query tile (identical for both parities; covers causal
# reach ceil((4i+5)/2), capped at the 16 local tiles)
CAPS = [min(NSB, 2 * i + 3) for i in range(NQT)]
MASK_FROM = [2 * i for i in range(NQT)]  # sigma >= 2i may cross the diagonal
# The mask for tile (i, s) depends only on e = 2s - 4i in {0, 2, 4}:
# thr = 128*(2s+p) + r - 512i - 1 = 128*e + 128*p + r - 1.  Three masks total.
M_IDX = {(i, s): (2 * s - 4 * i) // 2
         for i in range(NQT) for s in range(MASK_FROM[i], CAPS[i])}
N_MASKED = 3
W0 = 480                       # live-column window start for singleton s-tiles
# (p0 singleton has exactly 1 live col (511); p1 singleton is fully dead)
FP = mybir.dt.float16
F32 = mybir.dt.float32
SCALE = 1.0 / H                # (H**-0.5) applied twice


def build_bass():
    nc = bacc.Bacc("TRN2", target_bir_lowering=False, num_devices=NCORES)
    # DRAM layouts are pre-transposed on host and chunk-major:
    # x[p, chunk, ct, col] = x.T[128*ct+p, 512*chunk+col], so each 512-col
    # chunk DMA moves 8KB contiguous per partition (128 fat descriptors)
    qT = nc.declare_dram_parameter("qT", [128, NQT, NCT, 512], FP, isOutput=False)
    kT = nc.declare_dram_parameter("kT", [128, NQT // 2, NCT, 512], FP, isOutput=False)
    vT = nc.declare_dram_parameter("vT", [128, NQT // 2, NCT, 512], FP, isOutput=False)
    # wq|wk|wv merged into one DMA; iota|ident|thr merged into one (all fp16)
    w3 = nc.declare_dram_parameter("w3", [128, 3, NCT, H], FP, isOutput=False)
    misc = nc.declare_dram_parameter("misc", [128, TQ + 64 + N_MASKED], FP,
                                     isOutput=False)
    out = nc.declare_dram_parameter("out", [H + 1, T], F32, isOutput=True)

    with ExitStack() as ctx:
        tc = ctx.enter_context(tile.TileContext(nc))
        singles = ctx.enter_context(tc.tile_pool(name="singles", bufs=1))
        pt_pool = ctx.enter_context(tc.tile_pool(name="pt", bufs=6))
        outsb_pool = ctx.enter_context(tc.tile_pool(name="outsb", bufs=6))
        stage_pool = ctx.enter_context(tc.tile_pool(name="stage", bufs=2))
        psum_s = ctx.enter_context(tc.tile_pool(name="psum_s", bufs=3, space="PSUM"))
        psum_o = ctx.enter_context(tc.tile_pool(name="psum_o", bufs=2, space="PSUM"))

        # SBUF-resident tiles
        misc_sb = singles.tile([128, TQ + 64 + N_MASKED], FP)
        iota_sb = misc_sb[:, 0:TQ]
        id_sb = misc_sb[0:64, TQ:TQ + 64]
        thr_sb = singles.tile([128, N_MASKED], F32)
        w3_sb = singles.tile([128, 3, NCT, H], FP)
        wq_sb = w3_sb[:, 0]
        wk_sb = w3_sb[:, 1]
        wv_sb = w3_sb[:, 2]
        q_sb = singles.tile([128, NQT, NCT, 512], FP)
        k_sb = singles.tile([128, NQT // 2, NCT, 512], FP)
        v_sb = singles.tile([128, NQT // 2, NCT, 512], FP)

        qp_sb = singles.tile([128, T], FP)        # Q^T [h, t], dup on parts 64-127
        kp_sb = singles.tile([128, TH], FP)       # K^T [h, s], dup on parts 64-127
        va_sb = singles.tile([128, NSB, H + 1], FP)  # V rows [s, h] + ones col
        masks_sb = singles.tile([128, N_MASKED, TQ], FP)

        # ---- DMA issue stream (sync engine), deadline order ----
        # Arrival pacing is ~2.85us/MB; the exp (ACT) stream is paced by q_i
        # arrivals early on, so q chunks go as early as k/v deadlines allow.
        def dq(c):
            nc.sync.dma_start(out=q_sb[:, c, :, :], in_=qT[:, c, :, :])

        def dk(c, c0=0, c1=512):
            nc.sync.dma_start(out=k_sb[:, c, :, c0:c1], in_=kT[:, c, :, c0:c1])

        def dv(c, c0=0, c1=512):
            nc.sync.dma_start(out=v_sb[:, c, :, c0:c1], in_=vT[:, c, :, c0:c1])

        nc.sync.dma_start(out=w3_sb, in_=w3[:, :, :, :])
        dk(0)
        dq(0)
        dq(1)
        nc.sync.dma_start(out=misc_sb, in_=misc[:, :])
        dq(2)
        dk(1)
        dv(0)
        dq(3)
        dv(1)
        dq(4)
        dk(2)
        dq(5)
        dv(2)
        dk(3)
        dq(6)
        dq(7)
        dv(3)

        # ones column for the softmax denominator
        nc.vector.memset(va_sb[:, :, H:H + 1], 1.0)

        # PE warm-up: the first ~10us are DMA-dead (preamble + q0/k0 landing);
        # keep the PE busy on junk matmuls so HAM reaches K=8/8 (2.4 GHz)
        # before the first projection instead of ramping mid-projection.
        warm_sb = singles.tile([128, 576], FP)
        nc.vector.memset(warm_sb, 0.25)
        warm_ps = psum_s.tile([128, 512], F32, tag="ps")
        for _ in range(14):
            nc.tensor.matmul(warm_ps, warm_sb[:, 0:128], warm_sb[:, 64:576],
                             start=True, stop=True, skip_group_check=True)



        def build_masks():
            # Only 3 distinct masks exist (e = 2s-4i in {0,2,4}); build once.
            nc.vector.tensor_copy(            # fp16 -> f32 (is_ge wants f32)
                thr_sb, misc_sb[:, TQ + 64:TQ + 64 + N_MASKED])
            for m in range(N_MASKED):
                nc.vector.tensor_scalar(
                    masks_sb[:, m, :], iota_sb[:, :], thr_sb[:, m:m + 1],
                    None, mybir.AluOpType.is_ge)

        def q_proj(tq):
            pq = psum_s.tile([128, 512], F32, tag="ps")
            for ct in range(NCT):
                nc.tensor.matmul(pq[0:64, :], wq_sb[:, ct, :],
                                 q_sb[:, tq, ct, :], tile_position=(0, 0),
                                 start=(ct == 0), stop=(ct == NCT - 1))
                nc.tensor.matmul(pq[64:128, :], wq_sb[:, ct, :],
                                 q_sb[:, tq, ct, :], tile_position=(0, 64),
                                 start=(ct == 0), stop=(ct == NCT - 1),
                                 skip_group_check=True)
            nc.vector.tensor_copy(qp_sb[:, ts(tq, 512)], pq)

        def q_proj_pair(ta, tb):
            # Two q-tiles share one slot as independent col tiles; the
            # partition-dup (needed by the row-tiled scores) comes from DVE
            # copies instead of a redundant second matmul.
            pq = psum_s.tile([128, 512], F32, tag="ps")
            for ct in range(NCT):
                nc.tensor.matmul(pq[0:64, :], wq_sb[:, ct, :],
                                 q_sb[:, ta, ct, :], tile_position=(0, 0),
                                 start=(ct == 0), stop=(ct == NCT - 1))
                nc.tensor.matmul(pq[64:128, :], wq_sb[:, ct, :],
                                 q_sb[:, tb, ct, :], tile_position=(0, 64),
                                 start=(ct == 0), stop=(ct == NCT - 1),
                                 skip_group_check=True)
            nc.vector.tensor_copy(qp_sb[0:64, ts(ta, 512)], pq[0:64, :])
            nc.vector.tensor_copy(qp_sb[64:128, ts(ta, 512)], pq[0:64, :])
            nc.vector.tensor_copy(qp_sb[0:64, ts(tb, 512)], pq[64:128, :])
            nc.vector.tensor_copy(qp_sb[64:128, ts(tb, 512)], pq[64:128, :])

        def k_proj(c4, c0=0, c1=512):
            pk = psum_s.tile([128, 512], F32, tag="ps")
            for ct in range(NCT):
                nc.tensor.matmul(pk[0:64, c0:c1], wk_sb[:, ct, :],
                                 k_sb[:, c4, ct, c0:c1], tile_position=(0, 0),
                                 start=(ct == 0), stop=(ct == NCT - 1))
                nc.tensor.matmul(pk[64:128, c0:c1], wk_sb[:, ct, :],
                                 k_sb[:, c4, ct, c0:c1], tile_position=(0, 64),
                                 start=(ct == 0), stop=(ct == NCT - 1),
                                 skip_group_check=True)
            nc.vector.tensor_copy(kp_sb[:, 512 * c4 + c0:512 * c4 + c1],
                                  pk[:, c0:c1])

        def v_proj(c4, j0=0, j1=4):
            cols = slice(128 * j0, 128 * j1)
            pv = psum_s.tile([64, 512], F32, tag="ps")
            for ct in range(NCT):
                nc.tensor.matmul(pv[:, cols], wv_sb[:, ct, :],
                                 v_sb[:, c4, ct, cols],
                                 start=(ct == 0), stop=(ct == NCT - 1))
            vt_stage = stage_pool.tile([64, 512], FP)
            nc.vector.tensor_copy(vt_stage[:, cols], pv[:, cols])
            for j in range(j0, j1):
                sig = c4 * 4 + j
                ptr = psum_o.tile([128, H], FP, tag="oacc")
                nc.tensor.transpose(ptr, vt_stage[:, ts(j, 128)], id_sb)
                nc.vector.tensor_copy(va_sb[:, sig, 0:H], ptr)

        # Global PV pipeline: entries are (emit_fn, pre_fn) closures; the PV
        # stream lags the global scores stream by LAG groups, crossing
        # attention boundaries so the PE never idles on an exp boundary.
        LAG = 4
        PEND = []

        def pump():
            emit_fn, pre_fn = PEND.pop(0)
            if pre_fn is not None:
                pre_fn()           # v-projection feeding this PV
            emit_fn()

        def attention(i, mid=None, mid_leads=None, pre_flush=None):
            cap = CAPS[i]
            po = psum_o.tile([H + 1, 512], F32, tag="oacc")
            # group list: leading full pairs, then the diagonal pair (needs
            # no new k-projection), then the nearly-dead singleton (trimmed
            # to cols [W0,512)) last — its k-projection (mid_leads) overlaps
            # the diagonal pair's exp.  The first PV writing po is a
            # full-width pair (PSUM zero-region start semantics).
            groups = []
            lead = cap if i == 7 else 2 * i + 2
            for g0 in range(0, lead, 2):
                groups.append(("pair", g0))
            if i < 7:
                groups.append(("single", cap - 1))
            def emit_s(kind, g0):
                # scores matmuls + exp + mask for one group; returns pt handle
                if kind == "pair":
                    ps = psum_s.tile([128, 1024], F32, tag="ps")
                    for g in (0, 1):
                        sig = g0 + g
                        nc.tensor.matmul(ps[:, ts(g, 512)],
                                         kp_sb[ts(g, 64), ts(sig, 128)],
                                         qp_sb[ts(g, 64), ts(i, 512)],
                                         tile_position=(64 * g, 0),
                                         start=True, stop=True)
                    pt = pt_pool.tile([128, 1024], FP)
                    nc.scalar.activation(pt, ps,
                                         mybir.ActivationFunctionType.Exp,
                                         scale=SCALE)
                else:  # singleton: full-width scores (PSUM zero-region rule),
                    # but exp/mask/PV trimmed to the live cols [W0:512)
                    sig = g0
                    ps = psum_s.tile([128, 512], F32, tag="ps")
                    nc.tensor.matmul(ps,
                                     kp_sb[0:64, ts(sig, 128)],
                                     qp_sb[0:64, ts(i, 512)],
                                     tile_position=(0, 0), start=True, stop=True)
                    pt = pt_pool.tile([128, 512], FP)
                    nc.scalar.activation(pt[:, W0:512], ps[:, W0:512],
                                         mybir.ActivationFunctionType.Exp,
                                         scale=SCALE)
                return pt

            def emit_mask(kind, g0, pt):
                # deferred to PV-pop time so these DVE ops never sit ahead of
                # projection evacuations in the DVE queue
                if kind == "pair":
                    for g in (0, 1):
                        sig = g0 + g
                        if sig >= MASK_FROM[i]:
                            m = M_IDX[(i, sig)]
                            nc.vector.tensor_mul(pt[:, ts(g, 512)],
                                                 pt[:, ts(g, 512)],
                                                 masks_sb[:, m, :])
                else:
                    m = M_IDX[(i, g0)]
                    nc.vector.tensor_mul(pt[:, W0:512], pt[:, W0:512],
                                         masks_sb[:, m, W0:512])

            state = {"first": True}

            def emit_pv(kind, g0, pt, last_grp):
                if kind == "pair":
                    for g in (0, 1):
                        sig = g0 + g
                        nc.tensor.matmul(po, va_sb[:, sig, :], pt[:, ts(g, 512)],
                                         start=state["first"],
                                         stop=(last_grp and g == 1))
                        state["first"] = False
                else:
                    nc.tensor.matmul(po[:, W0:512], va_sb[:, g0, :],
                                     pt[:, W0:512], start=state["first"],
                                     stop=last_grp)
                    state["first"] = False

            def finalize():
                osb = outsb_pool.tile([H + 1, 512], F32)
                nc.vector.tensor_copy(osb, po)
                # gpsimd SWDGE ring: keeps the sync HWDGE ring free for the
                # input stream (ring-full blocking serializes transfers)
                nc.gpsimd.dma_start(out=out[:, ts(i, 512)], in_=osb)

            n_groups = len(groups)
            for gi, (kind, g0) in enumerate(groups):
                if kind == "single" and mid_leads is not None:
                    # k-projection feeding this singleton (its DMA chunk
                    # arrives later than the lead pairs' data)
                    mid_leads()
                pt = emit_s(kind, g0)
                last = (gi == n_groups - 1)
                pre = pre_flush if gi == 0 else None

                def entry(kind=kind, g0=g0, pt=pt, last=last):
                    emit_mask(kind, g0, pt)
                    emit_pv(kind, g0, pt, last_grp=last)
                    if last:
                        finalize()
                PEND.append((entry, pre))
                while len(PEND) > LAG:
                    pump()
            if mid is not None:
                # next tile's projections go here so the PE chews them while
                # the ACT engine finishes this tile's trailing exps
                mid()

        # ---- compute schedule: deadline-aligned with the DMA stream ----
        # attention(i) consumes qp_i, kp s-tiles <= 2i+2, va s-tiles <= 2i+2.
        # Each attention's mid-hook carries upcoming projections so they
        # overlap that tile's trailing exp stream on the ACT engine.
        build_masks()
        k_proj(0)                  # s0-3
        q_proj(0)
        attention(0, mid=lambda: q_proj(1), pre_flush=lambda: v_proj(0))
        attention(1, mid_leads=lambda: k_proj(1), mid=lambda: q_proj(2),
                  pre_flush=lambda: v_proj(1, 0, 2))
        attention(2, mid=lambda: q_proj(3),
                  pre_flush=lambda: v_proj(1, 2, 4))
        attention(3, mid_leads=lambda: k_proj(2), mid=lambda: q_proj(4),
                  pre_flush=lambda: v_proj(2, 0, 2))
        attention(4, mid=lambda: q_proj(5),
                  pre_flush=lambda: v_proj(2, 2, 4))
        attention(5, mid_leads=lambda: k_proj(3), mid=lambda: q_proj(6),
                  pre_flush=lambda: v_proj(3, 0, 2))
        attention(6, mid=lambda: q_proj(7),
                  pre_flush=lambda: v_proj(3, 2, 4))
        attention(7)
        while PEND:                # drain the tail of the global PV pipeline
            pump()

    nc.compile()
    return nc


_NC = None


def _get_nc():
    global _NC
    if _NC is None:
        _NC = build_bass()
    return _NC


def _prep_core_inputs(q, k, v, Wq, Wk, Wv):
    f2 = np.float16

    def wprep(W):
        # SBUF layout [p, ct, h] = W.T[ct*128+p, h]
        return np.ascontiguousarray(W.T.reshape(NCT, 128, H).transpose(1, 0, 2)).astype(f2)

    def xprep(x):
        # [p, chunk, ct, col] = x.T[128*ct+p, 512*chunk+col]
        xt = x.T.astype(f2)                       # [C, T']
        nch = xt.shape[1] // 512
        return np.ascontiguousarray(
            xt.reshape(NCT, 128, nch, 512).transpose(1, 2, 0, 3))

    w3_h = np.ascontiguousarray(
        np.stack([wprep(Wq), wprep(Wk), wprep(Wv)], axis=1))

    r = np.arange(128)
    in_maps = []
    for j in range(NCORES):
        b, p = j // 2, j % 2
        rows = (np.arange(TH) // 128) * 256 + p * 128 + (np.arange(TH) % 128)
        qT_h = xprep(q[b])
        kT_h = xprep(k[b][rows])
        vT_h = xprep(v[b][rows])
        misc_h = np.zeros((128, TQ + 64 + N_MASKED), f2)
        misc_h[:, 0:TQ] = np.arange(TQ, dtype=np.float32)[None, :]
        misc_h[0:64, TQ:TQ + 64] = np.eye(64, dtype=f2)
        for m in range(N_MASKED):
            misc_h[:, TQ + 64 + m] = (256 * m + 128 * p + r - 1).astype(f2)
        in_maps.append({
            "qT": qT_h, "kT": kT_h, "vT": vT_h,
            "w3": w3_h, "misc": misc_h,
        })
    return in_maps


def _run(inputs, trace=False, trace_kwargs=None):
    nc = _get_nc()
    in_maps = _prep_core_inputs(
        inputs["q"], inputs["k"], inputs["v"],
        inputs["Wq"], inputs["Wk"], inputs["Wv"])
    res = run_bass_kernel_spmd(nc, in_maps, list(range(NCORES)), trace=trace,
                               **(trace_kwargs or {}))
    outs = [res.results[j]["out"] for j in range(NCORES)]
    y = np.empty((B, T, H), np.float32)
    for b in range(B):
        s = outs[2 * b] + outs[2 * b + 1]      # [H+1, T]
        y[b] = (s[:H] / s[H:H + 1]).T
    return y, res


def kernel(q, k, v, Wq, Wk, Wv):
    y, _ = _run({"q": np.asarray(q), "k": np.asarray(k), "v": np.asarray(v),
                 "Wq": np.asarray(Wq), "Wk": np.asarray(Wk), "Wv": np.asarray(Wv)})
    return y

